# revision 22
# baseline (speedup 1.0000x reference)
"""Trainium2 Bass kernel for BaseSegHead (dynamic 1x1-conv seg logits).

Computes, for full inputs:
    qry_feats = in_feats @ qry_w.T + qry_b                  [1200, 32]
    key_map   = einsum('oc,bchw->bohw', key_w, feat_map) + key_b
    logits    = einsum('bnc,bchw->bnhw', qry_feats.reshape(4,300,32), key_map)
    out       = logits.reshape(1200, 160, 160)

Sharding: 8 cores = 4 batch images x 2 spatial (H) halves. Core c handles
batch b = c//2, rows h*80:(h+1)*80. Each core reads feat_map[b,:,rows,:],
its 300 queries, and writes a [300, 80*160] output shard -- no cross-core
communication and no duplicated feat_map reads.

Precision: matmul operands are shipped/produced as fp16 (full-rate on the
PE array vs 2 half-rate passes for fp32; also halves DMA bytes); all
accumulation stays fp32 in PSUM. The fp32 logits are rounded to fp16 for
the output DMA and upcast on the host.

TensorE array tiling: the 128x128 PE array is 16 independent 32x32
sub-arrays. The key projection (M=32) runs 4-way column-tiled, packing
hw-tiles t=4c..4c+3 into the four partition bands of ONE PSUM bank, so a
single bias-activation drains four tiles at once. The main einsum (K=32)
runs 4-way row-tiled: band b = t%4 holds q and key_map operands on SBUF
partitions 32b..32b+31, quadrupling matmul throughput.
"""

import os
import sys

sys.path.insert(0, "/opt/trn_rl_repo")
os.environ.setdefault("MYCRO_LOCAL_CACHE", "1")

import numpy as np

BATCH = 4
N_PER = 300
IN_DIM = 256
KEY_DIM = 32
FH = FW = 160
HHALF = FH // 2            # 80 rows per core
HW = HHALF * FW            # 12800 spatial positions per core
N_CORES = 8

FT = 2560                  # free-dim size of big SBUF tiles (feat / out staging)
NFT = HW // FT             # 5 big tiles per 128-channel chunk
MMN = 512                  # matmul moving free size (one fp32 PSUM bank)
PER_FT = FT // MMN         # 5 matmuls per big tile
N_T = HW // MMN            # 25 hw-tiles
N_SLOTS = (N_T + 3) // 4   # 7 column slots in the banded key_map layout
N_CHUNKS = ((0, 128), (128, 128), (256, 44))   # query-row chunks (300 rows)
CPACK_W = 728              # fp16: qry_wT (64) + in_featsT (600) + key_wT (64)

_CACHE = {}


def build_nc():
    import concourse.bass as bass
    import concourse.bacc as bacc
    import concourse.mybir as mybir
    from concourse import tile

    f32 = mybir.dt.float32
    f16 = mybir.dt.float16
    Ident = mybir.ActivationFunctionType.Identity

    nc = bacc.Bacc("TRN2", target_bir_lowering=False, debug=False)

    featT = nc.dram_tensor("featT", [IN_DIM, HW], f16, kind="ExternalInput")
    cpack = nc.dram_tensor("cpack", [128, CPACK_W], f16, kind="ExternalInput")
    bpack = nc.dram_tensor("bpack", [128, 2], f32, kind="ExternalInput")
    out = nc.dram_tensor("out", [N_PER, HW], f16, kind="ExternalOutput")

    with tile.TileContext(nc) as tc:
        with (
            tc.tile_pool(name="const", bufs=1) as cpool,
            tc.tile_pool(name="fpool", bufs=2 * NFT) as fpool,
            tc.tile_pool(name="opool", bufs=4) as opool,
            tc.tile_pool(name="kmap", bufs=1) as kpool,
            tc.tile_pool(name="ps_small", bufs=2, space=bass.MemorySpace.PSUM) as ps_small,
            tc.tile_pool(name="ps_main", bufs=6, space=bass.MemorySpace.PSUM) as ps_main,
        ):
            # --- constant loads -------------------------------------------
            ct = cpool.tile([128, CPACK_W], f16, name="ct")
            nc.sync.dma_start(ct[:], cpack[:])
            qw = (ct[:, 0:32], ct[:, 32:64])
            inT = (ct[:, 64:364], ct[:, 364:664])
            kw = (ct[:, 664:696], ct[:, 696:728])
            bt = cpool.tile([128, 2], f32, name="bt")
            nc.sync.dma_start(bt[:], bpack[:])
            qb = bt[:, 0:1]        # qry_b replicated in all four bands
            kb = bt[:, 1:2]        # key_b replicated in all four bands

            F = [[None] * NFT for _ in range(2)]

            def load_pair(i):
                for d in range(2):
                    ft = fpool.tile([128, FT], f16, name=f"feat_{d}_{i}", tag="fbf")
                    nc.sync.dma_start(
                        ft[:], featT[d * 128:(d + 1) * 128, i * FT:(i + 1) * FT]
                    )
                    F[d][i] = ft

            for i in range(NFT):
                load_pair(i)

            # --- key_map: 4-way column-tiled, banded layout ---------------
            # hw-tile t lives on SBUF partitions 32*(t%4), columns
            # (t//4)*512... ; one [128,512] PSUM bank holds 4 hw-tiles and
            # is drained by a single bias-activation.
            key_map = kpool.tile([128, N_SLOTS * MMN], f16, name="key_map")

            def key_quad(cs):
                kp = ps_small.tile([128, MMN], f32, name=f"kp_{cs}", tag="kp")
                nb = min(4, N_T - 4 * cs)
                for b in range(nb):
                    t = 4 * cs + b
                    i, o = t // PER_FT, (t % PER_FT) * MMN
                    for d in range(2):
                        nc.tensor.matmul(
                            kp[32 * b:32 * b + 32, :],
                            kw[d],
                            F[d][i][:, o:o + MMN],
                            start=(d == 0),
                            stop=(d == 1),
                            tile_position=(0, 32 * b),
                        )
                p = 32 * nb
                nc.scalar.activation(
                    key_map[0:p, cs * MMN:(cs + 1) * MMN], kp[0:p, :], Ident,
                    bias=kb[0:p, :],
                )

            key_quad(0)

            # --- qry projection, 4-way column-tiled (4 band copies) -------
            qp = ps_small.tile([128, MMN], f32, name="qp", tag="kp")
            for b in range(4):
                for d in range(2):
                    nc.tensor.matmul(
                        qp[32 * b:32 * b + 32, 0:N_PER],
                        qw[d],
                        inT[d],
                        start=(d == 0),
                        stop=(d == 1),
                        tile_position=(0, 32 * b),
                    )
            q_sb = cpool.tile([128, N_PER], f16, name="q_sb")
            nc.scalar.activation(q_sb[:], qp[:, 0:N_PER], Ident, bias=qb)

            for cs in range(1, N_SLOTS):
                key_quad(cs)

            # --- main einsum: 4-way row-tiled over band b = t%4 -----------
            # i-major order: all three query chunks of column block i only
            # need the key quads covering hw-tiles 5i..5i+4, so the output
            # stream starts as soon as each feat pair lands instead of
            # after the whole key phase.
            for i in range(NFT):
                scalar_js = (1, 3) if i <= 1 else (0, 2, 4)
                for (n0, m) in N_CHUNKS:
                    ot = opool.tile([128, FT], f16, name=f"ot_{n0}_{i}", tag="obuf")
                    for j in range(PER_FT):
                        t = i * PER_FT + j
                        b, cs = t % 4, t // 4
                        mp = ps_main.tile([128, MMN], f32, name=f"mp_{n0}_{t}", tag="mp")
                        nc.tensor.matmul(
                            mp[:m, :],
                            q_sb[32 * b:32 * b + 32, n0:n0 + m],
                            key_map[32 * b:32 * b + 32, cs * MMN:(cs + 1) * MMN],
                            tile_position=(32 * b, 0),
                        )
                        if j in scalar_js:
                            nc.scalar.copy(ot[:m, j * MMN:(j + 1) * MMN], mp[:m, :])
                        else:
                            nc.vector.tensor_copy(ot[:m, j * MMN:(j + 1) * MMN], mp[:m, :])
                    nc.gpsimd.dma_start(out[n0:n0 + m, i * FT:(i + 1) * FT], ot[:m, :])

    nc.compile()
    return nc


def _get_nc():
    if "nc" not in _CACHE:
        _CACHE["nc"] = build_nc()
    return _CACHE["nc"]


def make_in_maps(in_feats, feat_map, qry_w, qry_b, key_b, key_w):
    qwT = qry_w.T.astype(np.float16)                          # [256, 32]
    kwT = key_w.T.astype(np.float16)                          # [256, 32]
    bpack = np.zeros((128, 2), np.float32)
    bpack[:, 0] = np.tile(qry_b, 4)
    bpack[:, 1] = np.tile(key_b, 4)
    in_maps = []
    for c in range(N_CORES):
        b, h = divmod(c, 2)
        ifT = in_feats[b * N_PER:(b + 1) * N_PER].T.astype(np.float16)
        cpack = np.zeros((128, CPACK_W), np.float16)
        cpack[:, 0:32] = qwT[0:128]
        cpack[:, 32:64] = qwT[128:256]
        cpack[:, 64:364] = ifT[0:128]
        cpack[:, 364:664] = ifT[128:256]
        cpack[:, 664:696] = kwT[0:128]
        cpack[:, 696:728] = kwT[128:256]
        in_maps.append({
            "featT": np.ascontiguousarray(
                feat_map[b, :, h * HHALF:(h + 1) * HHALF, :]
            ).reshape(IN_DIM, HW).astype(np.float16),
            "cpack": cpack,
            "bpack": bpack,
        })
    return in_maps


def kernel(**inputs):
    in_feats = np.asarray(inputs["in_feats"], dtype=np.float32)
    feat_map = np.asarray(inputs["feat_map"], dtype=np.float32)
    qry_w = np.asarray(inputs["qry_w"], dtype=np.float32)
    qry_b = np.asarray(inputs["qry_b"], dtype=np.float32)
    key_w = np.asarray(inputs["key_w"], dtype=np.float32)
    key_b = np.asarray(inputs["key_b"], dtype=np.float32)

    from concourse import bass_utils

    nc = _get_nc()
    in_maps = make_in_maps(in_feats, feat_map, qry_w, qry_b, key_b, key_w)
    trace = os.environ.get("SEG_KERNEL_TRACE", "0") == "1"
    res = bass_utils.run_bass_kernel_spmd(
        nc, in_maps, core_ids=list(range(N_CORES)), trace=trace
    )
    _CACHE["last_result"] = res

    out = np.empty((BATCH * N_PER, FH, FW), dtype=np.float32)
    for c in range(N_CORES):
        b, h = divmod(c, 2)
        out[b * N_PER:(b + 1) * N_PER, h * HHALF:(h + 1) * HHALF, :] = (
            res.results[c]["out"].astype(np.float32).reshape(N_PER, HHALF, FW)
        )
    return out


# revision 23
# speedup vs baseline: 1.0673x; 1.0673x over previous
"""Trainium2 Bass kernel for BaseSegHead (dynamic 1x1-conv seg logits).

Computes, for full inputs:
    qry_feats = in_feats @ qry_w.T + qry_b                  [1200, 32]
    key_map   = einsum('oc,bchw->bohw', key_w, feat_map) + key_b
    logits    = einsum('bnc,bchw->bnhw', qry_feats.reshape(4,300,32), key_map)
    out       = logits.reshape(1200, 160, 160)

Sharding: 8 cores = 4 batch images x 2 spatial (H) halves. Core c handles
batch b = c//2, rows h*80:(h+1)*80. Each core reads feat_map[b,:,rows,:],
its 300 queries, and writes a [300, 80*160] output shard -- no cross-core
communication and no duplicated feat_map reads.

Precision: matmul operands are shipped/produced as fp16 (full-rate on the
PE array vs 2 half-rate passes for fp32; also halves DMA bytes); all
accumulation stays fp32 in PSUM. The fp32 logits are rounded to fp16 for
the output DMA and upcast on the host.

TensorE array tiling: the 128x128 PE array is 16 independent 32x32
sub-arrays. The key projection (M=32) runs 4-way column-tiled, packing
hw-tiles t=4c..4c+3 into the four partition bands of ONE PSUM bank, so a
single bias-activation drains four tiles at once. The main einsum (K=32)
runs 4-way row-tiled: band b = t%4 holds q and key_map operands on SBUF
partitions 32b..32b+31, quadrupling matmul throughput.
"""

import os
import sys

sys.path.insert(0, "/opt/trn_rl_repo")
os.environ.setdefault("MYCRO_LOCAL_CACHE", "1")

import numpy as np

BATCH = 4
N_PER = 300
IN_DIM = 256
KEY_DIM = 32
FH = FW = 160
HHALF = FH // 2            # 80 rows per core
HW = HHALF * FW            # 12800 spatial positions per core
N_CORES = 8

FT = 2560                  # free-dim size of big SBUF tiles (feat / out staging)
NFT = HW // FT             # 5 big tiles per 128-channel chunk
MMN = 512                  # matmul moving free size (one fp32 PSUM bank)
PER_FT = FT // MMN         # 5 matmuls per big tile
N_T = HW // MMN            # 25 hw-tiles
N_SLOTS = (N_T + 3) // 4   # 7 column slots in the banded key_map layout
N_CHUNKS = ((0, 128), (128, 128), (256, 44))   # query-row chunks (300 rows)
CPACK_W = 728              # fp16: qry_wT (64) + in_featsT (600) + key_wT (64)

_CACHE = {}


def build_nc():
    import concourse.bass as bass
    import concourse.bacc as bacc
    import concourse.mybir as mybir
    from concourse import tile

    f32 = mybir.dt.float32
    f16 = mybir.dt.float16
    Ident = mybir.ActivationFunctionType.Identity

    nc = bacc.Bacc("TRN2", target_bir_lowering=False, debug=False)

    featT = nc.dram_tensor("featT", [IN_DIM, HW], f16, kind="ExternalInput")
    cpack = nc.dram_tensor("cpack", [128, CPACK_W], f16, kind="ExternalInput")
    bpack = nc.dram_tensor("bpack", [128, 2], f32, kind="ExternalInput")
    out = nc.dram_tensor("out", [N_PER, HW], f16, kind="ExternalOutput")

    with tile.TileContext(nc) as tc:
        with (
            tc.tile_pool(name="const", bufs=1) as cpool,
            tc.tile_pool(name="fpool", bufs=2 * NFT) as fpool,
            tc.tile_pool(name="opool", bufs=4) as opool,
            tc.tile_pool(name="kmap", bufs=1) as kpool,
            tc.tile_pool(name="ps_small", bufs=2, space=bass.MemorySpace.PSUM) as ps_small,
            tc.tile_pool(name="ps_main", bufs=6, space=bass.MemorySpace.PSUM) as ps_main,
        ):
            # --- constant loads -------------------------------------------
            ct = cpool.tile([128, CPACK_W], f16, name="ct")
            nc.sync.dma_start(ct[:], cpack[:])
            qw = (ct[:, 0:32], ct[:, 32:64])
            inT = (ct[:, 64:364], ct[:, 364:664])
            kw = (ct[:, 664:696], ct[:, 696:728])
            bt = cpool.tile([128, 2], f32, name="bt")
            nc.sync.dma_start(bt[:], bpack[:])
            qb = bt[:, 0:1]        # qry_b replicated in all four bands
            kb = bt[:, 1:2]        # key_b replicated in all four bands

            F = [[None] * NFT for _ in range(2)]

            def load_pair(i):
                for d in range(2):
                    ft = fpool.tile([128, FT], f16, name=f"feat_{d}_{i}", tag="fbf")
                    nc.sync.dma_start(
                        ft[:], featT[d * 128:(d + 1) * 128, i * FT:(i + 1) * FT]
                    )
                    F[d][i] = ft

            for i in range(NFT):
                load_pair(i)

            # --- key_map: 4-way column-tiled, banded layout ---------------
            # hw-tile t lives on SBUF partitions 32*(t%4), columns
            # (t//4)*512... ; one [128,512] PSUM bank holds 4 hw-tiles and
            # is drained by a single bias-activation.
            key_map = kpool.tile([128, N_SLOTS * MMN], f16, name="key_map")

            def key_quad(cs):
                kp = ps_small.tile([128, MMN], f32, name=f"kp_{cs}", tag="kp")
                nb = min(4, N_T - 4 * cs)
                for b in range(nb):
                    t = 4 * cs + b
                    i, o = t // PER_FT, (t % PER_FT) * MMN
                    for d in range(2):
                        nc.tensor.matmul(
                            kp[32 * b:32 * b + 32, :],
                            kw[d],
                            F[d][i][:, o:o + MMN],
                            start=(d == 0),
                            stop=(d == 1),
                            tile_position=(0, 32 * b),
                        )
                p = 32 * nb
                nc.scalar.activation(
                    key_map[0:p, cs * MMN:(cs + 1) * MMN], kp[0:p, :], Ident,
                    bias=kb[0:p, :],
                )

            key_quad(0)

            # --- qry projection, 4-way column-tiled (4 band copies) -------
            qp = ps_small.tile([128, MMN], f32, name="qp", tag="kp")
            for b in range(4):
                for d in range(2):
                    nc.tensor.matmul(
                        qp[32 * b:32 * b + 32, 0:N_PER],
                        qw[d],
                        inT[d],
                        start=(d == 0),
                        stop=(d == 1),
                        tile_position=(0, 32 * b),
                    )
            q_sb = cpool.tile([128, N_PER], f16, name="q_sb")
            nc.scalar.activation(q_sb[:], qp[:, 0:N_PER], Ident, bias=qb)

            for cs in range(1, N_SLOTS):
                key_quad(cs)

            # --- main einsum: 4-way row-tiled over band b = t%4 -----------
            # i-major order: all three query chunks of column block i only
            # need the key quads covering hw-tiles 5i..5i+4, so the output
            # stream starts as soon as each feat pair lands instead of
            # after the whole key phase.
            for i in range(NFT):
                scalar_js = (1, 3) if i <= 1 else (0, 2, 4)
                for (n0, m) in N_CHUNKS:
                    ot = opool.tile([128, FT], f16, name=f"ot_{n0}_{i}", tag="obuf")
                    for j in range(PER_FT):
                        t = i * PER_FT + j
                        b, cs = t % 4, t // 4
                        mp = ps_main.tile([128, MMN], f32, name=f"mp_{n0}_{t}", tag="mp")
                        nc.tensor.matmul(
                            mp[:m, :],
                            q_sb[32 * b:32 * b + 32, n0:n0 + m],
                            key_map[32 * b:32 * b + 32, cs * MMN:(cs + 1) * MMN],
                            tile_position=(32 * b, 0),
                        )
                        if j in scalar_js:
                            nc.scalar.copy(ot[:m, j * MMN:(j + 1) * MMN], mp[:m, :])
                        else:
                            nc.vector.tensor_copy(ot[:m, j * MMN:(j + 1) * MMN], mp[:m, :])
                    nc.sync.dma_start(out[n0:n0 + m, i * FT:(i + 1) * FT], ot[:m, :])

    nc.compile()
    return nc


def _get_nc():
    if "nc" not in _CACHE:
        _CACHE["nc"] = build_nc()
    return _CACHE["nc"]


def make_in_maps(in_feats, feat_map, qry_w, qry_b, key_b, key_w):
    qwT = qry_w.T.astype(np.float16)                          # [256, 32]
    kwT = key_w.T.astype(np.float16)                          # [256, 32]
    bpack = np.zeros((128, 2), np.float32)
    bpack[:, 0] = np.tile(qry_b, 4)
    bpack[:, 1] = np.tile(key_b, 4)
    in_maps = []
    for c in range(N_CORES):
        b, h = divmod(c, 2)
        ifT = in_feats[b * N_PER:(b + 1) * N_PER].T.astype(np.float16)
        cpack = np.zeros((128, CPACK_W), np.float16)
        cpack[:, 0:32] = qwT[0:128]
        cpack[:, 32:64] = qwT[128:256]
        cpack[:, 64:364] = ifT[0:128]
        cpack[:, 364:664] = ifT[128:256]
        cpack[:, 664:696] = kwT[0:128]
        cpack[:, 696:728] = kwT[128:256]
        in_maps.append({
            "featT": np.ascontiguousarray(
                feat_map[b, :, h * HHALF:(h + 1) * HHALF, :]
            ).reshape(IN_DIM, HW).astype(np.float16),
            "cpack": cpack,
            "bpack": bpack,
        })
    return in_maps


def kernel(**inputs):
    in_feats = np.asarray(inputs["in_feats"], dtype=np.float32)
    feat_map = np.asarray(inputs["feat_map"], dtype=np.float32)
    qry_w = np.asarray(inputs["qry_w"], dtype=np.float32)
    qry_b = np.asarray(inputs["qry_b"], dtype=np.float32)
    key_w = np.asarray(inputs["key_w"], dtype=np.float32)
    key_b = np.asarray(inputs["key_b"], dtype=np.float32)

    from concourse import bass_utils

    nc = _get_nc()
    in_maps = make_in_maps(in_feats, feat_map, qry_w, qry_b, key_b, key_w)
    trace = os.environ.get("SEG_KERNEL_TRACE", "0") == "1"
    res = bass_utils.run_bass_kernel_spmd(
        nc, in_maps, core_ids=list(range(N_CORES)), trace=trace
    )
    _CACHE["last_result"] = res

    out = np.empty((BATCH * N_PER, FH, FW), dtype=np.float32)
    for c in range(N_CORES):
        b, h = divmod(c, 2)
        out[b * N_PER:(b + 1) * N_PER, h * HHALF:(h + 1) * HHALF, :] = (
            res.results[c]["out"].astype(np.float32).reshape(N_PER, HHALF, FW)
        )
    return out
